# revision 66
# baseline (speedup 1.0000x reference)
"""GQA attention block (B=2, L=2048, D=4096, H=32, HKV=8, RoPE, causal) on 8
Trainium2 NeuronCores.

Sharding: core c -> batch b=c//4, head-group g=c%4 (8 Q heads + 2 KV heads per
core).  Each core computes x[b] @ wq_g/wk_g/wv_g projections, RoPE, causal
attention for its heads, and a partial output projection against its slice of
wo (row-sharded contraction).  The host sums the 4 partials per batch element.

All matmul operands are bf16 (fp32 PSUM accumulation); measured end-to-end
rel-err ~4.5e-3 against the fp32 reference, well under the 2e-2 gate.  bf16
halves HBM traffic and SBUF footprint vs fp32r at the same PE rate (1
cycle/row), which is what lets x/weights/K/V/o2 all stay resident or
single-pass.

Projections run full 32-tile contraction chains into [128,1024] PSUM tiles
(two 512-query chunks per chain pair, contraction ordered by x-quarter DMA
arrival), so RoPE runs once per tile over 1024 columns.  V is projected
directly in [keys, head-dim] layout (x-slices stationary, wv moving), no PE
transposes.  One accumulation chain per PSUM bank everywhere: start=True
clears the whole bank.

Scores are computed transposed, S^T = K^T.T @ Q^T, two key-tiles per
[128,1024] PSUM tile so one Act-engine exp covers both (the ~580-cycle
per-ACTIVATE overhead was the attention rate limiter).  Score emission runs
LOOK double-tiles ahead across head boundaries.  Causality: fully-masked key
tiles are skipped, diagonal tiles also skip their dead query columns
(partial-width moving operands), and the surviving wedge is zeroed post-exp
with gpsimd affine_select (full tiles ordered first so the PE has mask-free
work while gpsimd runs).  Softmax denominator: e-tiles are pair+quad-summed on the
vector engine (bf16) and hit the PE as ONE all-ones matmul per 4 key-tiles;
normalization is reciprocal_approx_fast + multiply.

The output projection for each finished l-chunk is interleaved between the
attention heads of the NEXT l-chunk, filling the exp-latency slack in the PE
stream; the last chunk runs as a tail that borrows the idle score-PSUM pool
for 8 concurrent chains.

Pipeline-balance details that bought the last ~20us (761 -> ~743):
- Startup rides ONE ring in strict priority order (w0, then x in 1MB
  half-quarters): descriptor order is the only bandwidth-priority control,
  spreading across rings just splits the shared ~430GB/s.
- The second l-chunk's V-projection chains are deferred into the first
  attention chunk of each x-half, which is Act/gpsimd-bound with no oproj
  filler; they are spread across the early head items (PSUM pool rotation
  constrains where each may be popped -- see comments).
- Causal zeroing is one 2D-pattern affine_select per score double-tile
  (pattern [[-128,2],[1,512]]), halving gpsimd ops.
- The softmax partition-reduction hits the PE as ONE ones-matmul per
  (head, l-chunk); quads accumulate in-place on DVE first.
- wo tiles are host-interleaved so each load is one contiguous
  4KB-per-partition read, round-robined over all three HWDGE rings (each
  queue saturates near ~110GB/s, packet-rate bound); output stores pair
  two chains into one [128,1024] staging tile and a single 2KB-packet
  store into a tiled outT (1KB store packets were eating ~40% of ring
  time and the store completion sits on the obp-rotation critical loop).
Fast-regime wall time ~743us vs 1308us for the fp32r baseline; runs land
at ~890us when the chip drops to the ~2.0GHz P0 power state.  fp8
(DoubleRow) was evaluated and rejected: e4m3's ~2.6%/tensor noise puts
every fp8-GEMM config at 2.7e-2+ measured rel-err vs the 2e-2 gate.
"""

import numpy as np
import ml_dtypes

import concourse.mybir as mybir
import concourse.tile as tile
from concourse import bacc, bass_utils

B, L, D = 2, 2048, 4096
H, HKV, HD = 32, 8, 128
NCORES = 8
GROUPS = 4                # head groups (cores per batch element)
QH = H // GROUPS          # 8 q heads per core
KVH = HKV // GROUPS       # 2 kv heads per core
LC = 512                  # l-chunk (matmul moving free dim)
NLC = L // LC             # 4
DT = D // 128             # 32 contraction tiles for projections
SCALE = 1.0 / float(np.sqrt(HD))
LOOK = 3                  # score double-tiles in flight ahead of PV

f32 = mybir.dt.float32
bf16 = mybir.dt.bfloat16
nbf16 = ml_dtypes.bfloat16


def build_nc():
    nc = bacc.Bacc(trn_type="TRN2")
    # host-pre-tiled operands: every DMA below is a contiguous read
    x_tl = nc.dram_tensor("x_tl", [8, 128, 8 * 1024], bf16, kind="ExternalInput")
    wq_tl = nc.dram_tensor("wq_tl", [QH, 128, DT * 128], bf16, kind="ExternalInput")
    wk_tl = nc.dram_tensor("wk_tl", [KVH, 128, DT * 128], bf16, kind="ExternalInput")
    wv_tl = nc.dram_tensor("wv_tl", [128, DT * 256], bf16, kind="ExternalInput")
    # wo pre-interleaved in slab pairs: [g, p, a, b] = wo tile for output
    # rows (2g+a)*128, so one wo_g load is a single contiguous 4KB-per-
    # partition read (the old "a p b -> p a b" gather moved 2KB chunks and
    # took ~10us/MB -- it rate-limited the whole phase-3 tail)
    wo_tl = nc.dram_tensor(
        "wo_tl", [DT // 2, 128, 2 * QH * 128], bf16, kind="ExternalInput"
    )
    cosT = nc.dram_tensor("cosT", [64, L], bf16, kind="ExternalInput")
    sinT = nc.dram_tensor("sinT", [64, L], bf16, kind="ExternalInput")
    ones128 = nc.dram_tensor("ones128", [128, 128], bf16, kind="ExternalInput")
    # tiled output: [g][p, lcp*1024 + sl*512 + c] = out[(2g+sl)*128+p,
    # lcp*512+c] -- each pair-store is one 2KB-per-partition contiguous DMA
    outT_tl = nc.dram_tensor(
        "outT_tl", [DT // 2, 128, NLC * 1024], bf16, kind="ExternalOutput"
    )

    with tile.TileContext(nc) as tc:
        with (
            tc.tile_pool(name="persist", bufs=1) as pp,
            tc.tile_pool(name="xp", bufs=1) as xp,
            tc.tile_pool(name="qp", bufs=1) as qp,
            tc.tile_pool(name="wp", bufs=2) as wp,
            tc.tile_pool(name="ep", bufs=4) as ep,
            tc.tile_pool(name="esp", bufs=3) as esp,
            tc.tile_pool(name="tp", bufs=1) as tp,
            tc.tile_pool(name="rp", bufs=1) as rp,
            tc.tile_pool(name="obp", bufs=3) as obp,
            tc.tile_pool(name="wop", bufs=3) as wop,
            tc.tile_pool(name="mm2", bufs=2, space="PSUM") as mm2,
            tc.tile_pool(name="pop", bufs=2, space="PSUM") as pop,
            tc.tile_pool(name="dpp", bufs=2, space="PSUM") as dpp,
        ):
            kT_t = {
                kv: pp.tile([128, L], bf16, tag=f"kT_{kv}", name=f"kT_{kv}")
                for kv in range(KVH)
            }
            v_t = {
                lc: pp.tile([128, 1024], bf16, tag=f"v_{lc}", name=f"v_{lc}")
                for lc in range(NLC)
            }
            o2 = pp.tile([128, QH, L], bf16, tag="o2", name="o2")
            cs2 = pp.tile([128, L], bf16)
            sn2 = pp.tile([128, L], bf16)
            o128 = pp.tile([128, 128], bf16)
            wv_t = pp.tile([128, DT, 256], bf16, tag="wv", name="wv")

            def _load_tables(eng):
                eng.dma_start(cs2[0:64, :], cosT.ap())
                eng.dma_start(cs2[64:128, :], cosT.ap())
                eng.dma_start(sn2[0:64, :], sinT.ap())
                eng.dma_start(sn2[64:128, :], sinT.ap())
                # rotate-half: out = q*cs2 + swap(q)*sn2 with sn2 = [-sin|sin]
                nc.vector.tensor_scalar_mul(sn2[0:64, :], sn2[0:64, :], -1.0)
                eng.dma_start(o128[:], ones128.ap())

            def oproj_items(lcp):
                """32 output-projection chains for finished l-chunk lcp, as
                deferred closures.  wo tiles round-robin all three HWDGE
                rings (each queue tops out near ~110 GB/s, packet-rate
                bound), gpsimd first since its queue is empty when the
                prefetch fires.  The two chains per wo tile share one
                [128,1024] staging tile stored as a single 2KB-per-partition
                DMA (1KB store packets were eating ~40% of ring time)."""
                tiles = {}

                def dma(g):
                    wo_g = wop.tile([128, 2 * QH * 128], bf16, tag="wo",
                                    name=f"wo_g{lcp}_{g}")
                    (nc.gpsimd, nc.sync, nc.scalar)[g % 3].dma_start(
                        wo_g[:], wo_tl.ap()[g]
                    )
                    tiles[g] = wo_g

                dma(0)
                dma(1)
                dma(2)
                obs = {}

                def step(nt, lcp=lcp):
                    g, sl = divmod(nt, 2)
                    wo_t = tiles[g]
                    pso = (pop if sl == 0 else dpp).tile(
                        [128, LC], f32, tag=("po" if sl == 0 else "pden")
                    )
                    for h in range(QH):
                        nc.tensor.matmul(
                            pso[:],
                            wo_t[:, sl * 1024 + h * 128:sl * 1024 + (h + 1) * 128],
                            o2[:, h, lcp * LC:(lcp + 1) * LC],
                            start=(h == 0), stop=(h == QH - 1),
                        )
                    if sl == 0:
                        ob = obp.tile([128, 1024], bf16, tag="ob")
                        obs[g] = ob
                    else:
                        ob = obs.pop(g)
                    # both copies on DVE: an Act copy would queue behind the
                    # next head's 1.07us exp and delay PSUM recycling
                    nc.vector.tensor_copy(ob[:, sl * LC:(sl + 1) * LC], pso[:])
                    if sl == 1:
                        nc.sync.dma_start(
                            outT_tl.ap()[g][:, lcp * 1024:(lcp + 1) * 1024],
                            ob[:],
                        )
                        tiles.pop(g)
                        if g + 3 < DT // 2:
                            dma(g + 3)

                return [
                    (lambda nt=nt: step(nt)) for nt in range(DT)
                ]

            for pi in range(2):
                lcs = [2 * pi, 2 * pi + 1]
                c0 = lcs[0] * LC                      # first column of the pair
                # x quarters 0,1 on the sync ring, 2,3 on scalar; quarters
                # 0/2 plus the first weight tile issue ahead of everything
                # else so chain 0 starts as early as the shared SDMA
                # bandwidth allows (~20us)
                w_pre = {}
                x_t = xp.tile([128, DT, 1024], bf16, tag="x")

                def _xq(quar, eng, pi=pi, x_t=x_t):
                    eng.dma_start(
                        x_t[:, quar * 8:(quar + 1) * 8, :],
                        x_tl.ap()[pi * 4 + quar].rearrange(
                            "p (a b) -> p a b", a=8
                        ),
                    )

                if pi == 0:
                    # Startup: ONE ring (descriptor order = priority; the
                    # rings share HBM bandwidth so spreading loses priority
                    # control).  w0 first so chain 0 launches off the first
                    # half-quarter; x in 1MB half-quarters so the chain's
                    # dt-consumption paces with arrivals.  Tables ride the
                    # (otherwise idle) scalar ring; wv is emitted at phase
                    # 1b so it queues behind all chain weights.
                    w_t0 = wp.tile([128, DT * 128], bf16, tag="w", name="w_t0")
                    nc.sync.dma_start(w_t0[:], wq_tl.ap()[0])
                    w_pre[0] = w_t0

                    def _xh(quar, h, x_t=x_t):
                        nc.sync.dma_start(
                            x_t[:, quar * 8 + h * 4:quar * 8 + (h + 1) * 4, :],
                            x_tl.ap()[pi * 4 + quar][
                                :, h * 4096:(h + 1) * 4096
                            ].rearrange("p (a b) -> p a b", a=4),
                        )

                    _xh(0, 0)
                    _xh(0, 1)
                    _xh(1, 0)
                    w_t1 = wp.tile([128, DT * 128], bf16, tag="w", name="w_t1")
                    nc.sync.dma_start(w_t1[:], wq_tl.ap()[1])
                    w_pre[1] = w_t1
                    _xh(1, 1)
                    _xh(2, 0)
                    _xh(2, 1)
                    _xh(3, 0)
                    _xh(3, 1)
                    _load_tables(nc.scalar)
                else:
                    _xq(0, nc.sync)
                    _xq(1, nc.sync)
                    _xq(2, nc.scalar)
                    _xq(3, nc.scalar)
                q_pr = qp.tile([128, QH, 1024], bf16, tag="q")
                # ---- phase 1a: q/k projections + RoPE ----
                for mi in range(QH + KVH):
                    kind = "q" if mi < QH else "k"
                    m = mi if mi < QH else mi - QH
                    if mi in w_pre:
                        w_t = w_pre.pop(mi)
                    else:
                        w_t = wp.tile([128, DT * 128], bf16, tag="w")
                        nc.sync.dma_start(
                            w_t[:],
                            (wq_tl.ap()[m] if kind == "q" else wk_tl.ap()[m]),
                        )
                    ps = mm2.tile([128, 1024], f32, tag="mm2")
                    dts = list(range(DT))
                    for di, dt in enumerate(dts):
                        for lci in range(2):
                            nc.tensor.matmul(
                                ps[:, lci * LC:(lci + 1) * LC],
                                w_t[:, dt * 128:(dt + 1) * 128],
                                x_t[:, dt, lci * LC:(lci + 1) * LC],
                                start=(di == 0), stop=(di == DT - 1),
                            )
                    csl = slice(c0, c0 + 1024)
                    if kind == "q":
                        dst = q_pr[:, m, :]
                    else:
                        dst = kT_t[m][:, c0:c0 + 1024]
                    # single bf16 tmp (2KB/part, frees 6KB vs two f32 tmps
                    # -- the room pays for wop bufs=4); extra bf16 rounding
                    # of the cos term costs ~1e-4 rel-err, budget is 2e-2
                    t2 = tp.tile([128, 1024], bf16, tag="t2")
                    nc.vector.tensor_mul(t2[0:64, :], ps[64:128, :], sn2[0:64, csl])
                    nc.vector.tensor_mul(t2[64:128, :], ps[0:64, :], sn2[64:128, csl])
                    nc.vector.tensor_mul(dst, ps[:], cs2[:, csl])
                    nc.vector.tensor_tensor(dst, dst, t2[:], mybir.AluOpType.add)
                # ---- phase 1b: v projection, [keys, hd] layout directly ----
                # one accumulation chain per PSUM bank: start=True clears the
                # whole bank, so chains must not share one
                # Only the FIRST l-chunk's v is computed here; the second
                # chunk's 4 chains are deferred into the first attention
                # chunk below, whose PE slack (Act/gpsimd-bound, no oproj
                # filler available) they fill.
                if pi == 0:
                    # wv queues on sync behind the chain weights; it arrives
                    # ~15us before these chains need it without stealing
                    # startup bandwidth from x/wq
                    nc.sync.dma_start(wv_t[:], wv_tl.ap())
                for l4h in range(2):                  # l4 pair (2*l4h, 2*l4h+1)
                    psv = mm2.tile([128, 1024], f32, tag="mm2")
                    for dt in range(DT):
                        for sub in range(2):
                            l4 = 2 * l4h + sub
                            nc.tensor.matmul(
                                psv[:, sub * 512:sub * 512 + 256],
                                x_t[:, dt, l4 * 128:(l4 + 1) * 128],
                                wv_t[:, dt, :],
                                start=(dt == 0), stop=(dt == DT - 1),
                            )
                    for sub in range(2):
                        l4 = 2 * l4h + sub
                        nc.vector.tensor_copy(
                            v_t[lcs[0]][:, l4 * 256:(l4 + 1) * 256],
                            psv[:, sub * 512:sub * 512 + 256],
                        )

                def v_sub(l4, pool, tag, x_t=x_t, lc1=lcs[1]):
                    # each chain emits as TWO half-chains popped between
                    # consecutive items: a full 3.5us chain pauses item
                    # consumption (emits are tied to items), drains the
                    # Act exp lookahead, and pays a ~2us pipeline-refill
                    # bubble per chain -- the halves keep scores flowing
                    st = {}

                    def half(h0):
                        def run():
                            if h0 == 0:
                                st["ps"] = pool.tile(
                                    [128, LC], f32, tag=tag,
                                    name=f"vsub_{lc1}_{l4}",
                                )
                            ps = st["ps"]
                            for dt in range(h0, h0 + DT // 2):
                                nc.tensor.matmul(
                                    ps[:, 0:256],
                                    x_t[:, dt, LC + l4 * 128:LC + (l4 + 1) * 128],
                                    wv_t[:, dt, :],
                                    start=(dt == 0), stop=(dt == DT - 1),
                                )
                            if h0 != 0:
                                nc.vector.tensor_copy(
                                    v_t[lc1][:, l4 * 256:(l4 + 1) * 256],
                                    ps[:, 0:256],
                                )
                        return run

                    return [half(0), half(DT // 2)]

                vq = (
                    v_sub(0, pop, "po") + v_sub(1, pop, "po")
                    + v_sub(2, dpp, "pden") + v_sub(3, dpp, "pden")
                )
                # ---- phase 2: causal attention for this pair's l-chunks,
                # with the PREVIOUS l-chunk's output-projection chains
                # interleaved between heads (fills the exp-latency slack) ----
                # Emit contexts for BOTH l-chunks are built first so the
                # second chunk's first LOOK score tiles can be emitted
                # inside the first chunk's tail items: the tail's PV
                # matmuls wait on Act, the PE queue is strict FIFO, and
                # the next chunk's scores have no dependency on this
                # chunk -- without the pre-emit the PE idles there and
                # HAM re-throttles the clock.
                ctxs = []
                for lci in range(2):
                    lc = lcs[lci]
                    njt = 4 * (lc + 1)
                    nde = njt // 2
                    # full double-tiles first: they give the PE mask-free
                    # work while gpsimd zeroes the diagonal tiles, whose
                    # PV/denominator consumers come last
                    jt_pairs = [(j, j + 1) for j in range(0, 4 * lc, 2)] + [
                        (4 * lc, 4 * lc + 1), (4 * lc + 2, 4 * lc + 3)
                    ]
                    items = [(h, di) for h in range(QH) for di in range(nde)]
                    e_tiles = {}

                    def emit(idx, lci=lci, lc=lc, jt_pairs=jt_pairs,
                             items=items, e_tiles=e_tiles, q_pr=q_pr):
                        h, di = items[idx]
                        kv = h // (QH // KVH)
                        psS = mm2.tile([128, 2, LC], f32, tag="mm2")
                        for sub, jt in enumerate(jt_pairs[di]):
                            # diagonal tiles: queries y < 128*dg are fully
                            # masked -- skip those moving columns.  The
                            # uncovered psS region is stale, but affine_select
                            # below zeroes exactly that region of e.
                            dg = jt - 4 * lc
                            y0 = 128 * dg if dg > 0 else 0
                            nc.tensor.matmul(
                                psS[:, sub, y0:LC],
                                kT_t[kv][:, jt * 128:(jt + 1) * 128],
                                q_pr[:, h, lci * LC + y0:(lci + 1) * LC],
                                start=True, stop=True,
                            )
                        e = ep.tile([128, 2, LC], bf16, tag="e")
                        nc.scalar.activation(
                            e[:], psS[:], mybir.ActivationFunctionType.Exp,
                            scale=SCALE,
                        )
                        dg0 = jt_pairs[di][0] - 4 * lc
                        if dg0 >= 0:
                            # causal: zero E where key j > query l, both subs
                            # in ONE gpsimd op (a diagonal pair's dgs are
                            # always dg0, dg0+1): keep iff
                            # value(p, sub, y) = -p - 128*(dg0+sub) + y >= 0
                            nc.gpsimd.affine_select(
                                out=e[:],
                                in_=e[:],
                                compare_op=mybir.AluOpType.is_ge,
                                fill=0.0,
                                base=-128 * dg0,
                                pattern=[[-128, 2], [1, LC]],
                                channel_multiplier=-1,
                            )
                        e_tiles[idx] = e

                    ctxs.append((lc, nde, jt_pairs, items, e_tiles, emit))

                ke_next = [0, 0]
                for lci in range(2):
                    lc, nde, jt_pairs, items, e_tiles, emit = ctxs[lci]
                    e_sums = {}
                    # lcp=0's wo tiles prefetch during lc0 itself (first
                    # wop generations -- no rotation waits), so lc1's first
                    # interleaved oproj step never stalls on its weights
                    if pi == 0 and lci == 0:
                        opq_pre = oproj_items(0)
                        opq = []
                    elif pi == 0 and lci == 1:
                        opq = opq_pre
                    else:
                        opq = oproj_items(lc - 1)
                    ke = ke_next[lci]
                    while ke < min(LOOK, len(items)):
                        emit(ke)
                        ke += 1
                    if lci == 0 and vq:
                        vq.pop(0)()       # deferred v chain: PE work while
                                          # Act exps the first score tiles
                    po = pden = None
                    eacc = None
                    for idx, (h, di) in enumerate(items):
                        kv = h // (QH // KVH)
                        if di == 0:
                            po = pop.tile([128, LC], f32, tag="po")
                            pden = dpp.tile([128, LC], f32, tag="pden")
                        if ke < len(items):
                            emit(ke)
                            ke += 1
                        e = e_tiles.pop(idx)
                        for sub, jt in enumerate(jt_pairs[di]):
                            first = di == 0 and sub == 0
                            last = di == nde - 1 and sub == 1
                            dg = jt - 4 * lc
                            y0 = 128 * dg if dg > 0 else 0
                            esl = e[:, sub, y0:LC]
                            nc.tensor.matmul(
                                po[:, y0:LC],
                                v_t[jt // 4][
                                    :,
                                    (jt % 4) * 256 + kv * 128:
                                    (jt % 4) * 256 + (kv + 1) * 128,
                                ],
                                esl, start=first, stop=last,
                            )
                        # denominator: sum 4 key-tiles on DVE (bf16), one
                        # ones-matmul per QUAD of key tiles
                        esum = esp.tile([128, LC], bf16, tag="es")
                        nc.vector.tensor_tensor(
                            esum[:], e[:, 0, :], e[:, 1, :],
                            mybir.AluOpType.add,
                        )
                        e_sums[di] = esum
                        if di % 2 == 1:
                            esq = esp.tile([128, LC], bf16, tag="es2")
                            nc.vector.tensor_tensor(
                                esq[:], e_sums.pop(di - 1)[:], e_sums.pop(di)[:],
                                mybir.AluOpType.add,
                            )
                            # running bf16 accumulator over quads: the
                            # partition reduction hits the PE as ONE
                            # ones-matmul per (head, l-chunk) instead of
                            # one per quad (~10us of PE saved).  Accumulate
                            # INTO the newest quad so the old accumulator
                            # is freed by the add that consumes it (no
                            # extra tiles, no rotation deadlock).
                            if eacc is not None:
                                nc.vector.tensor_tensor(
                                    esq[:], esq[:], eacc[:],
                                    mybir.AluOpType.add,
                                )
                            eacc = esq
                        if di == nde - 1:
                            nc.tensor.matmul(
                                pden[:], o128[:], eacc[:],
                                start=True, stop=True,
                            )
                            eacc = None
                            rec = rp.tile([128, LC], f32, tag="rec")
                            nc.vector.reciprocal_approx_fast(
                                out=rec[:], in_=pden[:]
                            )
                            nc.vector.tensor_mul(
                                o2[:, h, lc * LC:(lc + 1) * LC], po[:], rec[:]
                            )
                            for _ in range(4):
                                if opq:
                                    opq.pop(0)()
                        # remaining deferred v chains, spread across the
                        # early items so the whole Act/gpsimd-bound window
                        # keeps PE work (dumping them all at once left
                        # heads 1-7 of lc0 dry).  The last dpp-pool chain
                        # reuses pden-h0's bank, so its pop must follow
                        # h0's final pden reader in program order: idx 5
                        # >= nde-1 for lc0 (nde=2), and for lc2 (nde=6)
                        # the pop at idx 5 lands right after h0's rec.
                        if lci == 0 and vq and idx in (0, 1, 2, 4, 6, 8, 10):
                            vq.pop(0)()
                        # (cross-chunk pre-emit was tried here and measured
                        # ~4us WORSE: pulling the next chunk's score tiles
                        # ahead displaces this chunk's final exps in the
                        # Act FIFO, which gates the tail PVs anyway)
                    while opq:
                        opq.pop(0)()
            # ---- phase 3 tail: output projection for the last l-chunk.
            # mm2 is idle here (no scores), so its 4 banks host two extra
            # chains alongside po/pden -> 8 chain slots; the PSUM->SBUF
            # copies alternate DVE / Act so neither engine gates recycling.
            lcp = NLC - 1
            wo_tiles = {}

            def wdma(g):
                wo_g = wop.tile([128, 2 * QH * 128], bf16, tag="wo",
                                name=f"wo_tail{g}")
                (nc.gpsimd, nc.sync, nc.scalar)[g % 3].dma_start(
                    wo_g[:], wo_tl.ap()[g]
                )
                wo_tiles[g] = wo_g

            wdma(0)
            wdma(1)
            wdma(2)
            ps2 = None
            obs3 = {}
            for nt in range(DT):
                g, sl = divmod(nt, 2)
                wo_t = wo_tiles[g]
                kind = nt % 4
                if kind == 0:
                    ps2 = mm2.tile([128, 1024], f32, tag="mm2")
                    pso = ps2[:, 0:LC]
                elif kind == 1:
                    pso = ps2[:, LC:1024]
                elif kind == 2:
                    psoa = pop.tile([128, LC], f32, tag="po", name=f"psoa{nt}")
                    pso = psoa[:]
                else:
                    psob = dpp.tile([128, LC], f32, tag="pden", name=f"psob{nt}")
                    pso = psob[:]
                for h in range(QH):
                    nc.tensor.matmul(
                        pso,
                        wo_t[:, sl * 1024 + h * 128:sl * 1024 + (h + 1) * 128],
                        o2[:, h, lcp * LC:(lcp + 1) * LC],
                        start=(h == 0), stop=(h == QH - 1),
                    )
                if sl == 0:
                    ob = obp.tile([128, 1024], bf16, tag="ob")
                    obs3[g] = ob
                else:
                    ob = obs3.pop(g)
                # copies alternate DVE / Act (no exps here) so neither
                # engine gates PSUM recycling; one 2KB-packet store per pair
                if nt % 2 == 0:
                    nc.vector.tensor_copy(ob[:, 0:LC], pso)
                else:
                    nc.scalar.copy(ob[:, LC:1024], pso)
                if sl == 1:
                    (nc.sync if g % 2 == 0 else nc.scalar).dma_start(
                        outT_tl.ap()[g][:, lcp * 1024:(lcp + 1) * 1024],
                        ob[:],
                    )
                    wo_tiles.pop(g)
                    if g + 3 < DT // 2:
                        wdma(g + 3)
    nc.compile()
    return nc


_PERM = np.concatenate([np.arange(0, HD, 2), np.arange(1, HD, 2)])


def _tile_weight(wT):
    """[D, M] (transposed weight) -> [M//128, 128, 32*128] bf16 tiles:
    tile m[p, dt*128 + mc] = wT[dt*128 + p, m*128 + mc]."""
    Dd, M = wT.shape
    w = wT.reshape(DT, 128, M // 128, 128)            # [dt, p, m, mc]
    w = w.transpose(2, 1, 0, 3)                       # [m, p, dt, mc]
    return np.ascontiguousarray(
        w.reshape(M // 128, 128, DT * 128).astype(nbf16)
    )


def shard_inputs(x, wq, wk, wv, wo, cos, sin, mask):
    """Build the 8 per-core input maps (host pre-tiling)."""
    cosT = np.ascontiguousarray(cos.T.astype(nbf16))
    sinT = np.ascontiguousarray(sin.T.astype(nbf16))
    ones = np.ones((128, 128), nbf16)

    x_tls = []
    for b in range(B):
        xT = x[b].T.astype(np.float32)                # [D, L]
        xv = xT.reshape(4, 8, 128, NLC // 2, 1024)    # [quar, dt8, p, pair, col]
        xv = xv.transpose(3, 0, 2, 1, 4)              # [pair, quar, p, dt8, col]
        x_tls.append(
            np.ascontiguousarray(xv.reshape(8, 128, 8 * 1024).astype(nbf16))
        )

    def permute_rows(w):
        nh = w.shape[0] // HD
        wp_ = w.reshape(nh, HD, -1)[:, _PERM, :]
        return wp_.reshape(w.shape)

    in_maps = []
    for c in range(NCORES):
        b, g = divmod(c, GROUPS)
        wq_g = permute_rows(wq[QH * HD * g:QH * HD * (g + 1)])
        wk_g = permute_rows(wk[KVH * HD * g:KVH * HD * (g + 1)])
        wv_g = wv[KVH * HD * g:KVH * HD * (g + 1)]
        wo_g = wo[:, QH * HD * g:QH * HD * (g + 1)]
        # wv moving tiles: [p, dt*256 + c] = wv_g.T[dt*128+p, c]
        wv_tl = np.ascontiguousarray(
            wv_g.T.astype(np.float32).reshape(DT, 128, 256)
            .transpose(1, 0, 2).reshape(128, DT * 256).astype(nbf16)
        )
        # wo stationary tiles, slab-pair interleaved for contiguous loads:
        # [g][p, a, h*128+n] = wo_g[(2g+a)*128+n, h*128+p]
        wov = wo_g.astype(np.float32).reshape(DT, 128, QH, 128)  # [nt, n, h, p]
        wov = wov.transpose(0, 3, 2, 1)                          # [nt, p, h, n]
        wo_tl = np.ascontiguousarray(
            wov.reshape(DT // 2, 2, 128, QH * 128)
            .transpose(0, 2, 1, 3)                               # [g, p, a, hn]
            .reshape(DT // 2, 128, 2 * QH * 128)
            .astype(nbf16)
        )
        in_maps.append({
            "x_tl": x_tls[b],
            "wq_tl": _tile_weight(wq_g.T),
            "wk_tl": _tile_weight(wk_g.T),
            "wv_tl": wv_tl,
            "wo_tl": wo_tl,
            "cosT": cosT,
            "sinT": sinT,
            "ones128": ones,
        })
    return in_maps


def gather_output(results):
    out = np.zeros((B, L, D), np.float32)
    for c in range(NCORES):
        b = c // GROUPS
        a = results[c]["outT_tl"].astype(np.float32).reshape(
            DT // 2, 128, NLC, 2, LC
        )
        out[b] += a.transpose(2, 4, 0, 3, 1).reshape(L, D)
    return out


_nc_cache = {}


def _get_nc():
    if "nc" not in _nc_cache:
        _nc_cache["nc"] = build_nc()
    return _nc_cache["nc"]


def run_sharded(inputs, trace=False, tmpdir=None):
    nc = _get_nc()
    in_maps = shard_inputs(**inputs)
    res = bass_utils.run_bass_kernel_spmd(
        nc, in_maps, core_ids=list(range(NCORES)), trace=trace, tmpdir=tmpdir
    )
    return gather_output(res.results), res


def kernel(**inputs) -> np.ndarray:
    out, _ = run_sharded(inputs)
    return out

